# revision 1
# baseline (speedup 1.0000x reference)
"""Trainium2 kernel for nn_BatchedDTW — mixed bf16/fp8 stream.

kernel4 is DVE-bound (fp8 sub runs 1x). Streaming a tuned fraction of the
columns as bf16 lets DVE sub them at 2x, trading HBM bytes for DVE cycles:
cols [0, F16) arrive bf16, cols [F16, F) arrive fp8-e4m3.  With F16=1280
(31%), SD=256 squares on DVE, rest on ACT, all three of DMA (~3.9us),
DVE (~3.9us) and ACT (~3.9us) balance below kernel4's ~4.3us DVE bound.
sqrt stays software-pipelined one rep behind its square (kernel4's fix).
"""

from contextlib import ExitStack

import numpy as np
import ml_dtypes

import concourse.bass as bass
import concourse.mybir as mybir
from concourse.bass_utils import run_bass_kernel_spmd

N_CORES = 8
P = 128
C = 32
B, T, N = 4, 512, 64
ROWS = B * T * N // N_CORES   # 16384 rows per core
F = ROWS // 4                 # 4096 free cols per partition
F16 = 1280                    # cols streamed as bf16 (rest fp8)
F8 = F - F16
SL = 256                      # moving cols per matmul
NSL = F // SL                 # 16
SD = 256                      # cols squared on DVE (slice 0); rest ACT
KSETS = 4

_nc_cache = {}
_last_results = None


def _build(repeat=1, nbuf=None):
    if nbuf is None:
        nbuf = 2 if repeat > 1 else 1
    nc = bass.Bass()
    bf16 = mybir.dt.bfloat16
    f32 = mybir.dt.float32
    f8 = mybir.dt.float8e4
    z16_ext = nc.declare_dram_parameter("z16", [P, 2 * F16], bf16, isOutput=False)
    z8_ext = nc.declare_dram_parameter("z8", [P, 2 * F8], f8, isOutput=False)
    w_ext = nc.declare_dram_parameter("w", [P, 8 * 64], bf16, isOutput=False)
    out_ext = nc.declare_dram_parameter("out", [P, 1], f32, isOutput=True)

    ksets = min(repeat, KSETS)
    with ExitStack() as ctx:
        zt16 = ctx.enter_context(nc.sbuf_tensor([P, nbuf * 2 * F16], bf16))
        zt8 = ctx.enter_context(nc.sbuf_tensor([P, nbuf * 2 * F8], f8))
        df = ctx.enter_context(nc.sbuf_tensor([P, nbuf * F], bf16))
        sq = ctx.enter_context(nc.sbuf_tensor([P, nbuf * F], bf16))
        wt = ctx.enter_context(nc.sbuf_tensor([P, 8 * 64], bf16))
        acc = ctx.enter_context(nc.sbuf_tensor([P, nbuf], f32))
        ps = ctx.enter_context(nc.psum_tensor([P, nbuf * 2 * SL], f32))
        zsems = [ctx.enter_context(nc.semaphore(f"zsem{r}_{i}"))
                 for r in range(ksets) for i in range(2)]
        wsem = ctx.enter_context(nc.semaphore("wsem"))
        vsem = ctx.enter_context(nc.semaphore("vsem"))
        asem = ctx.enter_context(nc.semaphore("asem"))
        psem = ctx.enter_context(nc.semaphore("psem"))
        osem = ctx.enter_context(nc.semaphore("osem"))
        block = ctx.enter_context(nc.Block())

        def zs(r, ch):
            return zsems[(r % ksets) * 2 + ch]

        def z_done(r):
            return 16 * (r // ksets + 1)

        def o16(r):
            return (r % nbuf) * 2 * F16

        def o8(r):
            return (r % nbuf) * 2 * F8

        def foff(r):
            return (r % nbuf) * F

        def poff(r):
            return (r % nbuf) * 2 * SL

        # vsem: per rep sub16, sub8, mul
        def v_sub16_done(r):
            return 3 * r + 1

        def v_sub8_done(r):
            return 3 * r + 2

        def v_mul_done(r):
            return 3 * r + 3

        # asem (ACT pipelined): sq0, [sq1, sqrt0], [sq2, sqrt1], ..., sqrt(R-1)
        def a_sq_done(r):
            return 1 if r == 0 else 2 * r

        def a_sqrt_done(r):
            return 2 * repeat if r == repeat - 1 else 2 * r + 3

        def p_done(r, s):
            return NSL * r + s + 1

        @block.sync
        def _(sync):
            sync.dma_start(out=wt[:], in_=w_ext[:]).then_inc(wsem, 16)
            for r in range(repeat):
                if r >= nbuf:
                    sync.wait_ge(vsem, v_sub16_done(r - nbuf))
                sync.dma_start(
                    out=zt16[:, o16(r):o16(r) + 2 * F16],
                    in_=z16_ext[:],
                ).then_inc(zs(r, 0), 16)
                if r >= nbuf:
                    sync.wait_ge(vsem, v_sub8_done(r - nbuf))
                sync.dma_start(
                    out=zt8[:, o8(r):o8(r) + 2 * F8],
                    in_=z8_ext[:],
                ).then_inc(zs(r, 1), 16)

        @block.vector
        def _(vector):
            for r in range(repeat):
                vector.wait_ge(zs(r, 0), z_done(r))
                if r >= nbuf:
                    # WAR: df slot last read by ACT's square of rep r-nbuf
                    vector.wait_ge(asem, a_sq_done(r - nbuf))
                vector.tensor_sub(
                    df[:, foff(r):foff(r) + F16],
                    zt16[:, o16(r):o16(r) + F16],
                    zt16[:, o16(r) + F16:o16(r) + 2 * F16],
                ).then_inc(vsem, 1)
                vector.wait_ge(zs(r, 1), z_done(r))
                vector.tensor_sub(
                    df[:, foff(r) + F16:foff(r) + F],
                    zt8[:, o8(r):o8(r) + F8],
                    zt8[:, o8(r) + F8:o8(r) + 2 * F8],
                ).then_inc(vsem, 1)
                if r >= nbuf:
                    # WAR: sq slice 0 last read by PE matmul 0 of rep r-nbuf
                    vector.wait_ge(psem, p_done(r - nbuf, 0))
                vector.tensor_mul(
                    sq[:, foff(r):foff(r) + SD],
                    df[:, foff(r):foff(r) + SD],
                    df[:, foff(r):foff(r) + SD],
                ).then_inc(vsem, 1)

        @block.scalar
        def _(scalar):
            def emit_square(r):
                scalar.wait_ge(vsem, v_sub8_done(r))
                if r >= nbuf:
                    # WAR: sq cols [SD, F) last read by PE of rep r-nbuf
                    scalar.wait_ge(psem, p_done(r - nbuf, NSL - 1))
                scalar.square(
                    out=sq[:, foff(r) + SD:foff(r) + F],
                    in_=df[:, foff(r) + SD:foff(r) + F],
                ).then_inc(asem, 1)

            def emit_sqrt(r):
                scalar.wait_ge(psem, p_done(r, NSL - 1))
                scalar.activation(
                    out=ps[:, poff(r) + SL:poff(r) + 2 * SL],
                    in_=ps[:, poff(r):poff(r) + SL],
                    func=mybir.ActivationFunctionType.Sqrt,
                    accum_out=acc[:, r % nbuf:r % nbuf + 1],
                ).then_inc(asem, 1)

            for r in range(repeat):
                emit_square(r)
                if r >= 1:
                    emit_sqrt(r - 1)
            emit_sqrt(repeat - 1)
            scalar.wait_ge(asem, a_sqrt_done(repeat - 1))
            scalar.dma_start(
                out=out_ext[:],
                in_=acc[:, (repeat - 1) % nbuf:(repeat - 1) % nbuf + 1],
            ).then_inc(osem, 16)
            scalar.wait_ge(osem, 16)

        @block.tensor
        def _(tensor):
            tensor.wait_ge(wsem, 16)
            for r in range(repeat):
                for s in range(NSL):
                    h, i = s // 8, s % 8
                    if s == 0:
                        # producer: DVE's mul covers sq slice 0
                        tensor.wait_ge(vsem, v_mul_done(r))
                    elif s == 1:
                        # producer: ACT's square covers [SD, F)
                        tensor.wait_ge(asem, a_sq_done(r))
                    if i == 0 and r >= nbuf:
                        # WAR: psum half reset; sqrt(r-nbuf) must have read it
                        tensor.wait_ge(asem, a_sqrt_done(r - nbuf))
                    tensor.matmul(
                        out=ps[64 * h:64 * (h + 1), poff(r):poff(r) + SL],
                        lhsT=wt[:, 64 * i:64 * (i + 1)],
                        rhs=sq[:, foff(r) + s * SL:foff(r) + (s + 1) * SL],
                        start=(i == 0),
                        stop=(i == 7),
                    ).then_inc(psem, 1)
    return nc


def make_weights():
    w = np.zeros((P, 8 * 64), dtype=np.float32)
    k = np.arange(P)
    for i in range(8):
        w[k, 64 * i + 4 * i + k // C] = 1.0
    return w.astype(ml_dtypes.bfloat16)


def pack_inputs(X, Y):
    def to_parts(A):
        A = np.asarray(A, dtype=np.float32).reshape(N_CORES, F, 4, C)
        return A.transpose(0, 2, 3, 1).reshape(N_CORES, P, F)

    Xp, Yp = to_parts(X), to_parts(Y)
    Z16 = np.concatenate([Xp[:, :, :F16], Yp[:, :, :F16]], axis=2)
    Z8 = np.concatenate([Xp[:, :, F16:], Yp[:, :, F16:]], axis=2)
    return (Z16.astype(ml_dtypes.bfloat16), Z8.astype(ml_dtypes.float8_e4m3))


def kernel(X, Y, window=None, **_):
    global _nc_cache, _last_results
    Z16, Z8 = pack_inputs(X, Y)
    W = make_weights()
    if "k" not in _nc_cache:
        _nc_cache["k"] = _build()
    in_maps = [{"z16": Z16[k], "z8": Z8[k], "w": W} for k in range(N_CORES)]
    res = run_bass_kernel_spmd(_nc_cache["k"], in_maps, list(range(N_CORES)))
    _last_results = res
    partials = np.stack([r["out"] for r in res.results])
    total = partials.astype(np.float64).sum()
    return np.float32(total / (B * N))



# revision 2
# speedup vs baseline: 1.7841x; 1.7841x over previous
"""Trainium2 kernel for nn_BatchedDTW — PE-DoubleRow diff, decoupled (v4).

v3 fix: (1) diff psum rotates over 3 slots so only chunk 3 of a rep waits
on same-rep squares (v3's 2-slot rotation serialized PE behind the DVE
copy+mult chain -> 6.4us/rep); (2) DVE mult writes bf16 (2x mode) into a
separate sqb buffer, halving DVE cost; ACT keeps fp8 sqf.  Sums split to
match: 8 accumulating bf16 matmuls over sqb (chunks 0,2), 4 DoubleRow fp8
matmuls over sqf (chunks 1,3), both into one [32, 512] psum slot; one ACT
sqrt+accum per rep.  Expected ~3.0us/rep (DMA-bound 1.048MB @ ~360GB/s)
vs baseline ~3.97us.
"""

from contextlib import ExitStack

import numpy as np
import ml_dtypes

import concourse.bass as bass
import concourse.mybir as mybir
from concourse.bass_utils import run_bass_kernel_spmd

N_CORES = 8
P = 128
C = 32
B, T, N = 4, 512, 64
ROWS = B * T * N // N_CORES   # 16384 rows per core
F = ROWS // 4                 # 4096 cols per partition
MM = 512                      # cols per diff matmul (1 psum bank)
CH = 1024                     # cols per square chunk (= 2 diff matmuls)
ST = 256
KSETS = 4

_nc_cache = {}
_last_results = None


def _build(repeat=1, nbuf=None):
    if nbuf is None:
        nbuf = 2 if repeat > 1 else 1
    nc = bass.Bass()
    bf16 = mybir.dt.bfloat16
    f32 = mybir.dt.float32
    f8 = mybir.dt.float8e4
    DR = mybir.MatmulPerfMode.DoubleRow

    z_ext = nc.declare_dram_parameter("z", [P, 2 * F], f8, isOutput=False)
    wd_ext = nc.declare_dram_parameter("wd", [P, 2 * P], f8, isOutput=False)
    wa_ext = nc.declare_dram_parameter("wa", [P, 8 * 32], bf16, isOutput=False)
    ws_ext = nc.declare_dram_parameter("ws", [P, 4 * 64], f8, isOutput=False)
    out_ext = nc.declare_dram_parameter("out", [32, 1], f32, isOutput=True)

    ksets = min(repeat, KSETS)
    with ExitStack() as ctx:
        zt = ctx.enter_context(nc.sbuf_tensor([P, nbuf * 2 * F], f8))
        dfb = ctx.enter_context(nc.sbuf_tensor([P, CH], bf16))
        sqb = ctx.enter_context(nc.sbuf_tensor([P, nbuf * 2 * CH], bf16))
        sqf = ctx.enter_context(nc.sbuf_tensor([P, nbuf * 2 * CH], f8))
        wdt = ctx.enter_context(nc.sbuf_tensor([P, 2 * P], f8))
        wat = ctx.enter_context(nc.sbuf_tensor([P, 8 * 32], bf16))
        wst = ctx.enter_context(nc.sbuf_tensor([P, 4 * 64], f8))
        acc = ctx.enter_context(nc.sbuf_tensor([32, nbuf], f32))
        junk = ctx.enter_context(nc.sbuf_tensor([32, 2 * ST], bf16))
        psd = ctx.enter_context(nc.psum_tensor([P, 3 * CH], f32))
        pss = ctx.enter_context(nc.psum_tensor([32, nbuf * 2 * ST], f32))
        zsems = [ctx.enter_context(nc.semaphore(f"zsem{r}"))
                 for r in range(ksets)]
        wsem = ctx.enter_context(nc.semaphore("wsem"))
        pdsem = ctx.enter_context(nc.semaphore("pdsem"))
        pssem = ctx.enter_context(nc.semaphore("pssem"))
        vsem = ctx.enter_context(nc.semaphore("vsem"))
        aqsem = ctx.enter_context(nc.semaphore("aqsem"))
        artsem = ctx.enter_context(nc.semaphore("artsem"))
        osem = ctx.enter_context(nc.semaphore("osem"))
        block = ctx.enter_context(nc.Block())

        def zs(r):
            return zsems[r % ksets]

        def z_done(r):
            return 16 * (r // ksets + 1)

        def zoff(r):
            return (r % nbuf) * 2 * F

        def qoff(r):
            return (r % nbuf) * 2 * CH

        def soff(r):
            return (r % nbuf) * 2 * ST

        def pslot(r, c):
            return ((4 * r + c) % 3) * CH

        # pdsem: 8 diff matmuls / rep
        def pd_done(r, mm):
            return 8 * r + mm + 1

        # pssem: 12 sum matmuls / rep (grpA bf16: 8, grpB DR: 4)
        def ps_grpA(r):
            return 12 * r + 8

        def ps_grpB(r):
            return 12 * r + 12

        # vsem: copy0, mult0, copy2, mult2
        def v_mult0(r):
            return 4 * r + 2

        def v_mult2(r):
            return 4 * r + 4

        # aqsem: sq1, sq3
        def a_sq1(r):
            return 2 * r + 1

        def a_sq3(r):
            return 2 * r + 2

        def a_sqrt(r):
            return r + 1

        @block.sync
        def _(sync):
            sync.dma_start(out=wdt[:], in_=wd_ext[:]).then_inc(wsem, 16)
            sync.dma_start(out=wat[:], in_=wa_ext[:]).then_inc(wsem, 16)
            sync.dma_start(out=wst[:], in_=ws_ext[:]).then_inc(wsem, 16)
            for r in range(repeat):
                if r >= nbuf:
                    sync.wait_ge(pdsem, pd_done(r - nbuf, 7))
                sync.dma_start(
                    out=zt[:, zoff(r):zoff(r) + 2 * F],
                    in_=z_ext[:],
                ).then_inc(zs(r), 16)

        @block.tensor
        def _(tensor):
            tensor.wait_ge(wsem, 48)
            wd3 = wdt[:].rearrange("p (j m) -> p j m", j=2)

            def emit_diffs(r):
                tensor.wait_ge(zs(r), z_done(r))
                z3 = zt[:, zoff(r):zoff(r) + 2 * F].rearrange(
                    "p (j n) -> p j n", j=2)
                for mm in range(8):
                    c = mm // 2
                    if mm % 2 == 0:
                        # WAR on psd slot: occupant is chunk (4r+c-3)
                        if c == 0 and r >= 1:
                            tensor.wait_ge(aqsem, a_sq1(r - 1))
                        elif c == 1 and r >= 1:
                            tensor.wait_ge(vsem, v_mult2(r - 1))
                        elif c == 2 and r >= 1:
                            tensor.wait_ge(aqsem, a_sq3(r - 1))
                        elif c == 3:
                            tensor.wait_ge(vsem, v_mult0(r))
                    po = pslot(r, c) + (mm % 2) * MM
                    tensor.matmul(
                        out=psd[:, po:po + MM],
                        lhsT=wd3,
                        rhs=z3[:, :, mm * MM:(mm + 1) * MM],
                        start=True,
                        stop=True,
                        perf_mode=DR,
                    ).then_inc(pdsem, 1)

            def emit_sums(r):
                # grpA: 8 accumulating bf16 matmuls over sqb [.., 2*CH]
                for i in range(8):
                    if i == 0:
                        tensor.wait_ge(vsem, v_mult2(r))
                        if r >= nbuf:
                            tensor.wait_ge(artsem, a_sqrt(r - nbuf))
                    tensor.matmul(
                        out=pss[0:32, soff(r):soff(r) + ST],
                        lhsT=wat[:, 32 * i:32 * (i + 1)],
                        rhs=sqb[:, qoff(r) + ST * i:qoff(r) + ST * (i + 1)],
                        start=(i == 0),
                        stop=(i == 7),
                        skip_group_check=True,
                    ).then_inc(pssem, 1)
                # grpB: 4 DR fp8 matmuls over sqf (j stride CH)
                sq3 = sqf[:, qoff(r):qoff(r) + 2 * CH].rearrange(
                    "p (j n) -> p j n", j=2)
                for t in range(4):
                    if t == 0:
                        tensor.wait_ge(aqsem, a_sq3(r))
                    ws3 = wst[:, 64 * t:64 * (t + 1)].rearrange(
                        "p (j m) -> p j m", j=2)
                    tensor.matmul(
                        out=pss[0:32, soff(r) + ST:soff(r) + 2 * ST],
                        lhsT=ws3,
                        rhs=sq3[:, :, ST * t:ST * (t + 1)],
                        start=(t == 0),
                        stop=(t == 3),
                        perf_mode=DR,
                        skip_group_check=True,
                    ).then_inc(pssem, 1)

            # sums pipelined one rep behind diffs: PE never stalls on the
            # same rep's DVE/ACT squares
            for r in range(repeat):
                emit_diffs(r)
                if r >= 1:
                    emit_sums(r - 1)
            emit_sums(repeat - 1)

        @block.vector
        def _(vector):
            for r in range(repeat):
                for ci, c in enumerate((0, 2)):
                    po = pslot(r, c)
                    vector.wait_ge(pdsem, pd_done(r, 2 * c + 1))
                    vector.tensor_copy(dfb[:], psd[:, po:po + CH]).then_inc(
                        vsem, 1)
                    if ci == 0 and r >= nbuf:
                        # WAR: sqb read by grpA of rep r-nbuf
                        vector.wait_ge(pssem, ps_grpA(r - nbuf))
                    vector.tensor_mul(
                        sqb[:, qoff(r) + ci * CH:qoff(r) + (ci + 1) * CH],
                        dfb[:],
                        dfb[:],
                    ).then_inc(vsem, 1)

        @block.scalar
        def _(scalar):
            def emit_sq(r, c):
                ci = (c - 1) // 2
                scalar.wait_ge(pdsem, pd_done(r, 2 * c + 1))
                if ci == 0 and r >= nbuf:
                    # WAR: sqf read by grpB of rep r-nbuf
                    scalar.wait_ge(pssem, ps_grpB(r - nbuf))
                scalar.square(
                    out=sqf[:, qoff(r) + ci * CH:qoff(r) + (ci + 1) * CH],
                    in_=psd[:, pslot(r, c):pslot(r, c) + CH],
                ).then_inc(aqsem, 1)

            def emit_sqrt(r):
                scalar.wait_ge(pssem, ps_grpB(r))
                scalar.activation(
                    out=junk[:],
                    in_=pss[:, soff(r):soff(r) + 2 * ST],
                    func=mybir.ActivationFunctionType.Sqrt,
                    accum_out=acc[:, r % nbuf:r % nbuf + 1],
                ).then_inc(artsem, 1)

            for r in range(repeat):
                emit_sq(r, 1)
                if r >= 1:
                    emit_sqrt(r - 1)
                emit_sq(r, 3)
            emit_sqrt(repeat - 1)
            scalar.wait_ge(artsem, a_sqrt(repeat - 1))
            scalar.dma_start(
                out=out_ext[:],
                in_=acc[:, (repeat - 1) % nbuf:(repeat - 1) % nbuf + 1],
            ).then_inc(osem, 16)
            scalar.wait_ge(osem, 16)

    return nc


def make_wdiff():
    w = np.zeros((P, 2 * P), dtype=np.float32)
    k = np.arange(P)
    w[k, k] = 1.0
    w[k, P + k] = -1.0
    return w.astype(ml_dtypes.float8_e4m3)


def make_wa():
    # grpA slice i: out m = 4i + p//32
    w = np.zeros((P, 8, 32), dtype=np.float32)
    g = np.arange(P) // C
    for i in range(8):
        w[np.arange(P), i, 4 * i + g] = 1.0
    return w.reshape(P, 8 * 32).astype(ml_dtypes.bfloat16)


def make_wsum():
    # grpB slice t: out m = 8t + 4j + p//32  (j pairs sqf halves, stride CH)
    w = np.zeros((P, 4, 2, 32), dtype=np.float32)
    g = np.arange(P) // C
    for t in range(4):
        for j in range(2):
            w[np.arange(P), t, j, 8 * t + 4 * j + g] = 1.0
    return w.reshape(P, 4 * 64).astype(ml_dtypes.float8_e4m3)


def pack_inputs(X, Y):
    def to_parts(A):
        A = np.asarray(A, dtype=np.float32).reshape(N_CORES, F, 4, C)
        return A.transpose(0, 2, 3, 1).reshape(N_CORES, P, F)

    Z = np.concatenate([to_parts(X), to_parts(Y)], axis=2)
    return Z.astype(ml_dtypes.float8_e4m3)


def make_in_maps(X, Y):
    Z = pack_inputs(X, Y)
    wd = make_wdiff()
    wa = make_wa()
    ws = make_wsum()
    return [{"z": Z[k], "wd": wd, "wa": wa, "ws": ws} for k in range(N_CORES)]


def kernel(X, Y, window=None, **_):
    global _nc_cache, _last_results
    in_maps = make_in_maps(X, Y)
    if "k" not in _nc_cache:
        _nc_cache["k"] = _build()
    res = run_bass_kernel_spmd(_nc_cache["k"], in_maps, list(range(N_CORES)))
    _last_results = res
    partials = np.stack([r["out"] for r in res.results])
    total = partials.astype(np.float64).sum()
    return np.float32(total / (B * N))


# revision 4
# speedup vs baseline: 3.8407x; 2.1527x over previous
"""Trainium2 kernel for nn_BatchedDTW — PE-DoubleRow diff, decoupled (v4).

v5 = v4 + wider sum psum: grpA (bf16, unconstrained dst) writes psum
partitions [32:64) while grpB (DoubleRow, dst must start at 0) keeps
[0:32), so one sqrt covers [64, 256] (507ns) instead of [32, 512]
(720ns), easing ACT, the tightest engine.

v3 fix: (1) diff psum rotates over 3 slots so only chunk 3 of a rep waits
on same-rep squares (v3's 2-slot rotation serialized PE behind the DVE
copy+mult chain -> 6.4us/rep); (2) DVE mult writes bf16 (2x mode) into a
separate sqb buffer, halving DVE cost; ACT keeps fp8 sqf.  Sums split to
match: 8 accumulating bf16 matmuls over sqb (chunks 0,2), 4 DoubleRow fp8
matmuls over sqf (chunks 1,3), both into one [32, 512] psum slot; one ACT
sqrt+accum per rep.  Expected ~3.0us/rep (DMA-bound 1.048MB @ ~360GB/s)
vs baseline ~3.97us.
"""

from contextlib import ExitStack

import numpy as np
import ml_dtypes

import concourse.bass as bass
import concourse.mybir as mybir
from concourse.bass_utils import run_bass_kernel_spmd

N_CORES = 8
P = 128
C = 32
B, T, N = 4, 512, 64
ROWS = B * T * N // N_CORES   # 16384 rows per core
F = ROWS // 4                 # 4096 cols per partition
MM = 512                      # cols per diff matmul (1 psum bank)
CH = 1024                     # cols per square chunk (= 2 diff matmuls)
ST = 256
KSETS = 4

_nc_cache = {}
_last_results = None


def _build(repeat=1, nbuf=None):
    if nbuf is None:
        nbuf = 2 if repeat > 1 else 1
    nc = bass.Bass()
    bf16 = mybir.dt.bfloat16
    f32 = mybir.dt.float32
    f8 = mybir.dt.float8e4
    DR = mybir.MatmulPerfMode.DoubleRow

    z_ext = nc.declare_dram_parameter("z", [P, 2 * F], f8, isOutput=False)
    wd_ext = nc.declare_dram_parameter("wd", [P, 2 * P], f8, isOutput=False)
    wa_ext = nc.declare_dram_parameter("wa", [P, 8 * 32], bf16, isOutput=False)
    ws_ext = nc.declare_dram_parameter("ws", [P, 4 * 64], f8, isOutput=False)
    out_ext = nc.declare_dram_parameter("out", [64, 1], f32, isOutput=True)

    ksets = min(repeat, KSETS)
    with ExitStack() as ctx:
        zt = ctx.enter_context(nc.sbuf_tensor([P, nbuf * 2 * F], f8))
        dfb = ctx.enter_context(nc.sbuf_tensor([P, CH], bf16))
        sqb = ctx.enter_context(nc.sbuf_tensor([P, nbuf * 2 * CH], bf16))
        sqf = ctx.enter_context(nc.sbuf_tensor([P, nbuf * 2 * CH], f8))
        wdt = ctx.enter_context(nc.sbuf_tensor([P, 2 * P], f8))
        wat = ctx.enter_context(nc.sbuf_tensor([P, 8 * 32], bf16))
        wst = ctx.enter_context(nc.sbuf_tensor([P, 4 * 64], f8))
        acc = ctx.enter_context(nc.sbuf_tensor([64, nbuf], f32))
        junk = ctx.enter_context(nc.sbuf_tensor([64, ST], bf16))
        psd = ctx.enter_context(nc.psum_tensor([P, 3 * CH], f32))
        pss = ctx.enter_context(nc.psum_tensor([64, nbuf * 2 * ST], f32))
        zsems = [ctx.enter_context(nc.semaphore(f"zsem{r}"))
                 for r in range(ksets)]
        wsem = ctx.enter_context(nc.semaphore("wsem"))
        pdsem = ctx.enter_context(nc.semaphore("pdsem"))
        pssem = ctx.enter_context(nc.semaphore("pssem"))
        vsem = ctx.enter_context(nc.semaphore("vsem"))
        aqsem = ctx.enter_context(nc.semaphore("aqsem"))
        artsem = ctx.enter_context(nc.semaphore("artsem"))
        osem = ctx.enter_context(nc.semaphore("osem"))
        block = ctx.enter_context(nc.Block())

        def zs(r):
            return zsems[r % ksets]

        def z_done(r):
            return 16 * (r // ksets + 1)

        def zoff(r):
            return (r % nbuf) * 2 * F

        def qoff(r):
            return (r % nbuf) * 2 * CH

        def soff(r):
            # one full 2KB psum bank per slot: concurrent accumulation
            # groups + the sqrt read must not share a bank
            return (r % nbuf) * 2 * ST

        def pslot(r, c):
            return ((4 * r + c) % 3) * CH

        # pdsem: 8 diff matmuls / rep
        def pd_done(r, mm):
            return 8 * r + mm + 1

        # pssem: 12 sum matmuls / rep (grpA bf16: 8, grpB DR: 4)
        def ps_grpA(r):
            return 12 * r + 8

        def ps_grpB(r):
            return 12 * r + 12

        # vsem: copy0, mult0, copy2, mult2
        def v_mult0(r):
            return 4 * r + 2

        def v_mult2(r):
            return 4 * r + 4

        # aqsem: sq1, sq3
        def a_sq1(r):
            return 2 * r + 1

        def a_sq3(r):
            return 2 * r + 2

        def a_sqrt(r):
            return r + 1

        @block.sync
        def _(sync):
            sync.dma_start(out=wdt[:], in_=wd_ext[:]).then_inc(wsem, 16)
            sync.dma_start(out=wat[:], in_=wa_ext[:]).then_inc(wsem, 16)
            sync.dma_start(out=wst[:], in_=ws_ext[:]).then_inc(wsem, 16)
            for r in range(repeat):
                if r >= nbuf:
                    sync.wait_ge(pdsem, pd_done(r - nbuf, 7))
                sync.dma_start(
                    out=zt[:, zoff(r):zoff(r) + 2 * F],
                    in_=z_ext[:],
                ).then_inc(zs(r), 16)

        @block.tensor
        def _(tensor):
            tensor.wait_ge(wsem, 48)
            wd3 = wdt[:].rearrange("p (j m) -> p j m", j=2)

            def emit_diffs(r):
                tensor.wait_ge(zs(r), z_done(r))
                z3 = zt[:, zoff(r):zoff(r) + 2 * F].rearrange(
                    "p (j n) -> p j n", j=2)
                for mm in range(8):
                    c = mm // 2
                    if mm % 2 == 0:
                        # WAR on psd slot: occupant is chunk (4r+c-3)
                        if c == 0 and r >= 1:
                            tensor.wait_ge(aqsem, a_sq1(r - 1))
                        elif c == 1 and r >= 1:
                            tensor.wait_ge(vsem, v_mult2(r - 1))
                        elif c == 2 and r >= 1:
                            tensor.wait_ge(aqsem, a_sq3(r - 1))
                        elif c == 3:
                            tensor.wait_ge(vsem, v_mult0(r))
                    po = pslot(r, c) + (mm % 2) * MM
                    tensor.matmul(
                        out=psd[:, po:po + MM],
                        lhsT=wd3,
                        rhs=z3[:, :, mm * MM:(mm + 1) * MM],
                        start=True,
                        stop=True,
                        perf_mode=DR,
                    ).then_inc(pdsem, 1)

            def emit_sums(r):
                # grpA: 8 accumulating bf16 matmuls over sqb [.., 2*CH]
                for i in range(8):
                    if i == 0:
                        tensor.wait_ge(vsem, v_mult2(r))
                        if r >= nbuf:
                            tensor.wait_ge(artsem, a_sqrt(r - nbuf))
                    tensor.matmul(
                        out=pss[32:64, soff(r):soff(r) + ST],
                        lhsT=wat[:, 32 * i:32 * (i + 1)],
                        rhs=sqb[:, qoff(r) + ST * i:qoff(r) + ST * (i + 1)],
                        start=(i == 0),
                        stop=(i == 7),
                        skip_group_check=True,
                    ).then_inc(pssem, 1)
                # grpB: 4 DR fp8 matmuls over sqf (j stride CH)
                sq3 = sqf[:, qoff(r):qoff(r) + 2 * CH].rearrange(
                    "p (j n) -> p j n", j=2)
                for t in range(4):
                    if t == 0:
                        tensor.wait_ge(aqsem, a_sq3(r))
                    ws3 = wst[:, 64 * t:64 * (t + 1)].rearrange(
                        "p (j m) -> p j m", j=2)
                    tensor.matmul(
                        out=pss[0:32, soff(r):soff(r) + ST],
                        lhsT=ws3,
                        rhs=sq3[:, :, ST * t:ST * (t + 1)],
                        start=(t == 0),
                        stop=(t == 3),
                        perf_mode=DR,
                        skip_group_check=True,
                    ).then_inc(pssem, 1)

            # sums pipelined one rep behind diffs: PE never stalls on the
            # same rep's DVE/ACT squares
            for r in range(repeat):
                emit_diffs(r)
                if r >= 1:
                    emit_sums(r - 1)
            emit_sums(repeat - 1)

        @block.vector
        def _(vector):
            for r in range(repeat):
                for ci, c in enumerate((0, 2)):
                    po = pslot(r, c)
                    vector.wait_ge(pdsem, pd_done(r, 2 * c + 1))
                    vector.tensor_copy(dfb[:], psd[:, po:po + CH]).then_inc(
                        vsem, 1)
                    if ci == 0 and r >= nbuf:
                        # WAR: sqb read by grpA of rep r-nbuf
                        vector.wait_ge(pssem, ps_grpA(r - nbuf))
                    vector.tensor_mul(
                        sqb[:, qoff(r) + ci * CH:qoff(r) + (ci + 1) * CH],
                        dfb[:],
                        dfb[:],
                    ).then_inc(vsem, 1)

        @block.scalar
        def _(scalar):
            def emit_sq(r, c):
                ci = (c - 1) // 2
                scalar.wait_ge(pdsem, pd_done(r, 2 * c + 1))
                if ci == 0 and r >= nbuf:
                    # WAR: sqf read by grpB of rep r-nbuf
                    scalar.wait_ge(pssem, ps_grpB(r - nbuf))
                scalar.square(
                    out=sqf[:, qoff(r) + ci * CH:qoff(r) + (ci + 1) * CH],
                    in_=psd[:, pslot(r, c):pslot(r, c) + CH],
                ).then_inc(aqsem, 1)

            def emit_sqrt(r):
                scalar.wait_ge(pssem, ps_grpB(r))
                scalar.activation(
                    out=junk[:],
                    in_=pss[:, soff(r):soff(r) + ST],
                    func=mybir.ActivationFunctionType.Sqrt,
                    accum_out=acc[:, r % nbuf:r % nbuf + 1],
                ).then_inc(artsem, 1)

            for r in range(repeat):
                emit_sq(r, 1)
                if r >= 1:
                    emit_sqrt(r - 1)
                emit_sq(r, 3)
            emit_sqrt(repeat - 1)
            scalar.wait_ge(artsem, a_sqrt(repeat - 1))
            scalar.dma_start(
                out=out_ext[:],
                in_=acc[:, (repeat - 1) % nbuf:(repeat - 1) % nbuf + 1],
            ).then_inc(osem, 16)
            scalar.wait_ge(osem, 16)

    return nc


def make_wdiff():
    w = np.zeros((P, 2 * P), dtype=np.float32)
    k = np.arange(P)
    w[k, k] = 1.0
    w[k, P + k] = -1.0
    return w.astype(ml_dtypes.float8_e4m3)


def make_wa():
    # grpA slice i: out m = 4i + p//32
    w = np.zeros((P, 8, 32), dtype=np.float32)
    g = np.arange(P) // C
    for i in range(8):
        w[np.arange(P), i, 4 * i + g] = 1.0
    return w.reshape(P, 8 * 32).astype(ml_dtypes.bfloat16)


def make_wsum():
    # grpB slice t: out m = 8t + 4j + p//32  (j pairs sqf halves, stride CH)
    w = np.zeros((P, 4, 2, 32), dtype=np.float32)
    g = np.arange(P) // C
    for t in range(4):
        for j in range(2):
            w[np.arange(P), t, j, 8 * t + 4 * j + g] = 1.0
    return w.reshape(P, 4 * 64).astype(ml_dtypes.float8_e4m3)


def pack_inputs(X, Y):
    def to_parts(A):
        A = np.asarray(A, dtype=np.float32).reshape(N_CORES, F, 4, C)
        return A.transpose(0, 2, 3, 1).reshape(N_CORES, P, F)

    Z = np.concatenate([to_parts(X), to_parts(Y)], axis=2)
    return Z.astype(ml_dtypes.float8_e4m3)


def make_in_maps(X, Y):
    Z = pack_inputs(X, Y)
    wd = make_wdiff()
    wa = make_wa()
    ws = make_wsum()
    return [{"z": Z[k], "wd": wd, "wa": wa, "ws": ws} for k in range(N_CORES)]


def kernel(X, Y, window=None, **_):
    global _nc_cache, _last_results
    in_maps = make_in_maps(X, Y)
    if "k" not in _nc_cache:
        _nc_cache["k"] = _build()
    res = run_bass_kernel_spmd(_nc_cache["k"], in_maps, list(range(N_CORES)))
    _last_results = res
    partials = np.stack([r["out"] for r in res.results])
    total = partials.astype(np.float64).sum()
    return np.float32(total / (B * N))
